# revision 1
# baseline (speedup 1.0000x reference)
"""Trainium2 Bass kernel for nn_Block_69578470195514 (dense transformer block).

Contract: kernel(**inputs) takes the FULL unsharded inputs (B=8,T=1024,D=1024,
H=16) as numpy arrays and returns the FULL [8,1024,1024] float32 output.

Sharding: pure data-parallel over batch — core b processes batch element b.
Weights are replicated. No collectives.

Per-core dataflow (all matmuls bf16 with fp32 PSUM accumulation):
  LN1 (t-major, bn_stats)  ->  xc bf16
  PE-transpose xc -> xnT [d,t]
  qT = Wq^T xnT, kT = Wk^T xnT   (per-head [e,t] layout, heads concat on rows)
  v  = xnT^T Wv                  (natural [s,e] layout)
  per head: scoresT[s,t] = kT^T... lhsT=kT chunk, rhs=qT  (causal: only t>=s-tile)
            mask diag block, exp on ACT (scale=D^-0.5) with accum_out row sums
            r = 1/sum; v' = v*r; attT[e,t] += v'^T wexp (PSUM accumulation)
  proj: x2 = x + attT^T Wp (+bp)
  LN2 -> xc2 -> transpose -> xn2T
  hT = relu(W1^T xn2T + b1)      (bias via ACT relu bias, gamma2/beta2 folded)
  out = x2 + hT^T W2 (+b2)
"""

import numpy as np
import ml_dtypes

BF16 = ml_dtypes.bfloat16

P = 128
B, T, D, H = 8, 1024, 1024, 16
DH = D // H
F = 4 * D
NT = T // P   # 8 token tiles
ND = D // P   # 8 feature tiles
NF = F // P   # 32 ff tiles
EPS = 1e-3
SCALE = float(D) ** -0.5
MASKVAL = -1.0e6

_cache = {}


def _split_multiwait_insts(nc, mybir):
    """This walrus build allows only 1 sync-wait per instruction. Tile can
    attach several. Hoist all but the last wait of any instruction into
    preceding single-wait InstEventSemaphore carriers on the same engine
    (equivalent: the engine stalls at each carrier before dispatching)."""
    for bb in nc.main_func.blocks:
        insts = list(bb.instructions)
        out = []
        changed = False
        for inst in insts:
            si = inst.sync_info
            if si is not None and si.on_wait and len(si.on_wait) > 1:
                waits = list(si.on_wait)
                for k, w in enumerate(waits[:-1]):
                    d = mybir.InstEventSemaphore(
                        name=f"{inst.name}_wsplit{k}", ins=[], outs=[]
                    )
                    d.engine = inst.engine
                    d.sync_info = mybir.SyncInfo(on_wait=[w], on_update=[])
                    out.append(d)
                inst.sync_info = mybir.SyncInfo(
                    on_wait=[waits[-1]], on_update=list(si.on_update)
                )
                changed = True
            out.append(inst)
        if changed:
            try:
                bb.instructions[:] = out
            except Exception:
                bb.instructions.clear()
                for i in out:
                    bb.add_instruction(i)


def _av_chunks(r0):
    """Column chunks for the AV/score matmuls of s-tile starting at r0,
    split on PSUM bank boundaries (512 fp32)."""
    chunks = []
    for b0 in range(0, T, 512):
        lo = max(r0, b0)
        hi = b0 + 512
        if lo < hi:
            chunks.append((lo, hi))
    return chunks


def _build(reps=1, has_bp=False, has_b2=False, debug=False):
    from contextlib import ExitStack

    import concourse.bass as bass
    import concourse.tile as tile
    import concourse.mybir as mybir

    f32 = mybir.dt.float32
    bf16 = mybir.dt.bfloat16
    AF = mybir.ActivationFunctionType
    ALU = mybir.AluOpType

    nc = bass.Bass()

    x_d = nc.dram_tensor("x", [T, D], f32, kind="ExternalInput")
    wq_d = nc.dram_tensor("wq", [D, D], bf16, kind="ExternalInput")
    wk_d = nc.dram_tensor("wk", [D, D], bf16, kind="ExternalInput")
    wv_d = nc.dram_tensor("wv", [D, D], bf16, kind="ExternalInput")
    wp_d = nc.dram_tensor("wp", [D, D], bf16, kind="ExternalInput")
    w1_d = nc.dram_tensor("w1", [D, F], bf16, kind="ExternalInput")
    w2_d = nc.dram_tensor("w2", [F, D], bf16, kind="ExternalInput")
    b1_d = nc.dram_tensor("b1t", [P, NF], f32, kind="ExternalInput")
    qb_d = nc.dram_tensor("qbt", [P, ND], f32, kind="ExternalInput")
    kb_d = nc.dram_tensor("kbt", [P, ND], f32, kind="ExternalInput")
    vb_d = nc.dram_tensor("vbr", [1, D], f32, kind="ExternalInput")
    mask_d = nc.dram_tensor("mask", [P, P], f32, kind="ExternalInput")
    id_d = nc.dram_tensor("ident", [P, P], bf16, kind="ExternalInput")
    if has_bp:
        bp_d = nc.dram_tensor("bpr", [1, D], f32, kind="ExternalInput")
    if has_b2:
        b2_d = nc.dram_tensor("b2r", [1, D], f32, kind="ExternalInput")
    out_d = nc.dram_tensor("out", [T, D], f32, kind="ExternalOutput")
    dbg = {}
    if debug:
        for nm, shp in [
            ("d_xnT", [D, T]),
            ("d_qT", [D, T]),
            ("d_kT", [D, T]),
            ("d_v", [T, D]),
            ("d_attT", [D, T]),
            ("d_x2", [T, D]),
            ("d_h", [F, T]),
        ]:
            dbg[nm] = nc.dram_tensor(nm, shp, f32, kind="ExternalOutput")

    def bcast(ap_1d):
        # [1, N] dram row -> broadcast across partitions
        return bass.AP(
            tensor=ap_1d.tensor,
            offset=ap_1d.offset,
            ap=[[0, P]] + list(ap_1d.ap)[1:],
        )

    with tile.TileContext(nc, pool_alloc_mode="queue") as tc, ExitStack() as top:
        const = top.enter_context(tc.tile_pool(name="const", bufs=1))
        mask_sb = const.tile([P, P], f32)
        id_sb = const.tile([P, P], bf16)
        b1_sb = const.tile([P, NF], f32)
        qb_sb = const.tile([P, ND], f32)
        kb_sb = const.tile([P, ND], f32)
        vb_sb = const.tile([P, D], f32)
        eps_sb = const.tile([P, 1], f32)
        nc.vector.memset(eps_sb, EPS)
        bp_sb = b2_sb = None
        if has_bp:
            bp_sb = const.tile([P, D], f32)
        if has_b2:
            b2_sb = const.tile([P, D], f32)

        def const_dmas():
            # issued after the first x-tile loads: the identity is needed by
            # the first transpose ~5us in; nothing else until QKV/attention
            nc.sync.dma_start(out=id_sb, in_=id_d[:, :])
            nc.sync.dma_start(out=mask_sb, in_=mask_d[:, :])
            nc.sync.dma_start(out=b1_sb, in_=b1_d[:, :])
            nc.sync.dma_start(out=qb_sb, in_=qb_d[:, :])
            nc.sync.dma_start(out=kb_sb, in_=kb_d[:, :])
            nc.sync.dma_start(out=vb_sb, in_=bcast(vb_d[:, :]))
            if bp_sb is not None:
                nc.sync.dma_start(out=bp_sb, in_=bcast(bp_d[:, :]))
            if b2_sb is not None:
                nc.sync.dma_start(out=b2_sb, in_=bcast(b2_d[:, :]))

        emit_args = (
            nc, tc, tile, bass, mybir, f32, bf16, AF, ALU,
            x_d, wq_d, wk_d, wv_d, wp_d, w1_d, w2_d, out_d, dbg,
            mask_sb, id_sb, b1_sb, qb_sb, kb_sb, vb_sb, eps_sb,
            bp_sb, b2_sb,
            reps == 1, const_dmas,
        )
        if reps == 1:
            _emit(*emit_args)
        else:
            with tc.For_i(0, reps, 1):
                _emit(*emit_args)

    _split_multiwait_insts(nc, mybir)
    return nc


def _emit(
    nc, tc, tile, bass, mybir, f32, bf16, AF, ALU,
    x_d, wq_d, wk_d, wv_d, wp_d, w1_d, w2_d, out_d, dbg,
    mask_sb, id_sb, b1_sb, qb_sb, kb_sb, vb_sb, eps_sb, bp_sb, b2_sb,
    use_swdge=True, const_dmas=None,
):
    from contextlib import ExitStack

    big_dma = nc.gpsimd if use_swdge else nc.sync

    def ln_tile(stats, xin, xcout, tags):
        st = stats.tile([P, 2, 6], f32, tag=tags + "st")
        nc.vector.bn_stats(out=st[:, 0, :], in_=xin[:, 0:512])
        nc.vector.bn_stats(out=st[:, 1, :], in_=xin[:, 512:1024])
        mv = stats.tile([P, 2], f32, tag=tags + "mv")
        nc.vector.bn_aggr(out=mv, in_=st)
        sd = stats.tile([P, 1], f32, tag=tags + "sd")
        nc.scalar.activation(sd, mv[:, 1:2], AF.Sqrt, bias=eps_sb)
        rs = stats.tile([P, 1], f32, tag=tags + "rs")
        nc.vector.reciprocal(out=rs, in_=sd)
        nmu = stats.tile([P, 1], f32, tag=tags + "nmu")
        nc.vector.tensor_scalar(
            out=nmu, in0=mv[:, 0:1], scalar1=rs, scalar2=-1.0,
            op0=ALU.mult, op1=ALU.mult,
        )
        # (x - mu) * rsig on ACT: Identity(x*rs + (-mu*rs)); frees DVE
        nc.scalar.activation(xcout, xin, AF.Identity, bias=nmu, scale=rs)

    with ExitStack() as ctx:
        # Long-lived arrays on the RIGHT allocation stack (independent LIFO).
        pR1 = ctx.enter_context(tc.tile_pool(name="pR1", bufs=1, side="right"))
        x2 = pR1.tile([P, NT, D], f32)        # residual stream 2 [t, d]
        pR2 = ctx.enter_context(tc.tile_pool(name="pR2", bufs=1, side="right"))
        xn2T = pR2.tile([P, ND, T], bf16)     # ln2(x2)^T [d, t]

        # ======== phases A..C scope ========
        with ExitStack() as pab:
            pA = pab.enter_context(tc.tile_pool(name="pA", bufs=1))
            xnT = pA.tile([P, ND, T], bf16)   # xc^T  [d, t]
            qT = pA.tile([P, ND, T], bf16)    # [e, t]
            kT = pA.tile([P, ND, T], bf16)    # [e, s]
            v = pA.tile([P, NT, D], bf16)     # [s, e]
            pBt = pab.enter_context(
                tc.tile_pool(name="pBt", bufs=1, side="right")
            )
            attT = pBt.tile([P, ND, T], bf16)  # [e, t], pair p rows

            # wqk/psM span phase A (hoisted pairs) + attention
            qkscope = ExitStack()
            wqk = qkscope.enter_context(tc.tile_pool(name="wqk", bufs=3))
            psM = qkscope.enter_context(
                tc.tile_pool(name="psM", bufs=2, space="PSUM")
            )
            wq_ap = wq_d[:, :].rearrange("(k p) e -> p k e", p=P)
            wk_ap = wk_d[:, :].rearrange("(k p) e -> p k e", p=P)

            def qk_proj(pr):
                for wap, dest, bias_sb in (
                    (wq_ap, qT, qb_sb),
                    (wk_ap, kT, kb_sb),
                ):
                    wt = wqk.tile([P, ND, P], bf16, tag="wt")
                    nc.sync.dma_start(
                        out=wt, in_=wap[:, :, P * pr : P * (pr + 1)]
                    )
                    for n in range(2):
                        ps = psM.tile([P, 512], f32, tag="mm")
                        for k in range(ND):
                            nc.tensor.matmul(
                                ps,
                                wt[:, k, :],
                                xnT[:, k, 512 * n : 512 * (n + 1)],
                                start=(k == 0),
                                stop=(k == ND - 1),
                            )
                        nc.vector.tensor_scalar_add(
                            out=dest[:, pr, 512 * n : 512 * (n + 1)],
                            in0=ps,
                            scalar1=bias_sb[:, pr : pr + 1],
                        )

            # ============ Phase A: LN1 + transpose + QKV ============
            with ExitStack() as pa:
                xload = pa.enter_context(tc.tile_pool(name="xload", bufs=4))
                stats = pa.enter_context(tc.tile_pool(name="stats", bufs=6))
                xcp = pa.enter_context(tc.tile_pool(name="xcp", bufs=2))
                psT = pa.enter_context(
                    tc.tile_pool(name="psT", bufs=4, space="PSUM")
                )
                wvp = pa.enter_context(tc.tile_pool(name="wvp", bufs=1))
                psV = pa.enter_context(
                    tc.tile_pool(name="psV", bufs=2, space="PSUM")
                )

                xts = []
                for i in range(NT):
                    xt = xload.tile([P, D], f32, tag="xt")
                    nc.sync.dma_start(out=xt, in_=x_d[P * i : P * (i + 1), :])
                    xts.append(xt)
                    if i == 1 and const_dmas is not None:
                        const_dmas()
                for i in range(NT):
                    xt = xts[i]
                    xc = xcp.tile([P, D], bf16, tag="xc")
                    ln_tile(stats, xt, xc, "a")
                    for j in range(ND):
                        tp = psT.tile([P, P], bf16, tag="tp")
                        nc.tensor.transpose(
                            tp, xc[:, P * j : P * (j + 1)], id_sb
                        )
                        dst = xnT[:, j, P * i : P * (i + 1)]
                        if j % 2 == 0:
                            nc.vector.tensor_copy(out=dst, in_=tp)
                        else:
                            nc.scalar.copy(out=dst, in_=tp)

                qk_proj(0)
                qk_proj(1)
                wv_sb = wvp.tile([P, ND, D], bf16)
                wv_ap = wv_d[:, :].rearrange("(k p) e -> p k e", p=P)
                for k in range(ND):
                    big_dma.dma_start(
                        out=wv_sb[:, k, :], in_=wv_ap[:, k, :]
                    )
                for m in range(NT):
                    for n in range(2):
                        ps = psV.tile([P, 512], f32, tag="mmv")
                        for k in range(ND):
                            nc.tensor.matmul(
                                ps,
                                xnT[:, k, P * m : P * (m + 1)],
                                wv_sb[:, k, 512 * n : 512 * (n + 1)],
                                start=(k == 0),
                                stop=(k == ND - 1),
                            )
                        nc.vector.tensor_add(
                            out=v[:, m, 512 * n : 512 * (n + 1)],
                            in0=ps,
                            in1=vb_sb[:, 512 * n : 512 * (n + 1)],
                        )

                if dbg:
                    for j in range(ND):
                        nc.gpsimd.dma_start(
                            out=dbg["d_xnT"][P * j : P * (j + 1), :],
                            in_=xnT[:, j, :],
                        )
                        nc.gpsimd.dma_start(
                            out=dbg["d_qT"][P * j : P * (j + 1), :],
                            in_=qT[:, j, :],
                        )
                        nc.gpsimd.dma_start(
                            out=dbg["d_kT"][P * j : P * (j + 1), :],
                            in_=kT[:, j, :],
                        )
                        nc.gpsimd.dma_start(
                            out=dbg["d_v"][P * j : P * (j + 1), :],
                            in_=v[:, j, :],
                        )

            # ========= Phase B: per-pair q/k projections + attention =========
            with ExitStack() as pb:
                wexpp = pb.enter_context(tc.tile_pool(name="wexpp", bufs=1))
                asml = pb.enter_context(tc.tile_pool(name="asml", bufs=3))
                psS = pb.enter_context(
                    tc.tile_pool(name="psS", bufs=1, space="PSUM")
                )
                psA = pb.enter_context(
                    tc.tile_pool(name="psA", bufs=1, space="PSUM")
                )
                for pr in range(ND):  # head pairs
                    if pr >= 2:
                        qk_proj(pr)
                    # Both heads of a pair interleaved per s-tile: their score
                    # matmuls target different PE row-groups (K=64 at bases
                    # 0/64) and their AV matmuls different col-groups, so
                    # adjacent instructions run concurrently in the array.
                    attps = psA.tile([P, T], f32, tag="att")
                    wexpA = wexpp.tile([P, NT, T], bf16, tag="wexpA")
                    wexpB = wexpp.tile([P, NT, T], bf16, tag="wexpB")
                    wexps = [wexpA, wexpB]
                    sums = asml.tile([P, 2, NT], f32, tag="sums")
                    rr = asml.tile([P, 2, NT], f32, tag="rr")
                    for i in range(NT):
                        r0 = P * i
                        rlen = T - r0
                        spsA = psS.tile([P, rlen], f32, tag="scA")
                        spsB = psS.tile([P, rlen], f32, tag="scB")
                        sps2 = [spsA, spsB]
                        for c0 in range(0, rlen, 512):
                            cl = min(512, rlen - c0)
                            for hb in range(2):
                                base = 64 * hb
                                nc.tensor.matmul(
                                    sps2[hb][:, c0 : c0 + cl],
                                    kT[base : base + 64, pr, r0 : r0 + P],
                                    qT[
                                        base : base + 64,
                                        pr,
                                        r0 + c0 : r0 + c0 + cl,
                                    ],
                                    start=True,
                                    stop=True,
                                    tile_position=(base, 0),
                                )
                        for hb in range(2):
                            nc.vector.tensor_add(
                                out=sps2[hb][:, 0:P],
                                in0=sps2[hb][:, 0:P],
                                in1=mask_sb,
                            )
                        for hb in range(2):
                            nc.scalar.activation(
                                wexps[hb][:, i, r0:T],
                                sps2[hb],
                                AF.Exp,
                                scale=SCALE,
                                accum_out=sums[:, hb, i : i + 1],
                            )
                        nc.vector.reciprocal(
                            out=rr[:, :, i : i + 1], in_=sums[:, :, i : i + 1]
                        )
                        vp = asml.tile([P, 2, 64], bf16, tag="vp")
                        for hb in range(2):
                            base = 64 * hb
                            nc.vector.tensor_scalar_mul(
                                out=vp[:, hb, :],
                                in0=v[:, i, P * pr + base : P * pr + base + 64],
                                scalar1=rr[:, hb, i : i + 1],
                            )
                        for lo, hi in _av_chunks(r0):
                            bank = lo // 512
                            last_i = min(NT - 1, 4 * bank + 3)
                            for hb in range(2):
                                base = 64 * hb
                                nc.tensor.matmul(
                                    attps[base : base + 64, lo:hi],
                                    vp[:, hb, :],
                                    wexps[hb][:, i, lo:hi],
                                    start=(i == 0),
                                    stop=(i == last_i),
                                    tile_position=(0, base),
                                )
                    nc.vector.tensor_copy(out=attT[:, pr, :], in_=attps)

            qkscope.close()

            if dbg:
                for j in range(ND):
                    nc.gpsimd.dma_start(
                        out=dbg["d_attT"][P * j : P * (j + 1), :],
                        in_=attT[:, j, :],
                    )

            # ============ Phase C: proj + residual + LN2 ============
            psT2 = ctx.enter_context(
                tc.tile_pool(name="psT2", bufs=4, space="PSUM")
            )
            psM2 = ctx.enter_context(
                tc.tile_pool(name="psM2", bufs=3, space="PSUM")
            )
            with ExitStack() as pc:
                wpp = pc.enter_context(tc.tile_pool(name="wpp", bufs=1))
                xre = pc.enter_context(tc.tile_pool(name="xre", bufs=3))
                stats2 = pc.enter_context(tc.tile_pool(name="stats2", bufs=4))
                xcp2 = pc.enter_context(tc.tile_pool(name="xcp2", bufs=2))

                wp_sb = wpp.tile([P, ND, D], bf16)
                wp_ap = wp_d[:, :].rearrange("(k p) e -> p k e", p=P)
                for k in range(ND):
                    big_dma.dma_start(
                        out=wp_sb[:, k, :], in_=wp_ap[:, k, :]
                    )
                for m in range(NT):
                    for n in range(2):
                        pps = psM2.tile([P, 512], f32, tag="mm2")
                        for k in range(ND):
                            nc.tensor.matmul(
                                pps,
                                attT[:, k, P * m : P * (m + 1)],
                                wp_sb[:, k, 512 * n : 512 * (n + 1)],
                                start=(k == 0),
                                stop=(k == ND - 1),
                            )
                        xt = xre.tile([P, 512], f32, tag="xre")
                        nc.sync.dma_start(
                            out=xt,
                            in_=x_d[
                                P * m : P * (m + 1), 512 * n : 512 * (n + 1)
                            ],
                        )
                        nc.vector.tensor_add(
                            out=x2[:, m, 512 * n : 512 * (n + 1)],
                            in0=xt,
                            in1=pps,
                        )
                        if bp_sb is not None:
                            nc.vector.tensor_add(
                                out=x2[:, m, 512 * n : 512 * (n + 1)],
                                in0=x2[:, m, 512 * n : 512 * (n + 1)],
                                in1=bp_sb[:, 512 * n : 512 * (n + 1)],
                            )
                    xc2 = xcp2.tile([P, D], bf16, tag="xc2")
                    ln_tile(stats2, x2[:, m, :], xc2, "c")
                    for j in range(ND):
                        tp = psT2.tile([P, P], bf16, tag="tp2")
                        nc.tensor.transpose(
                            tp, xc2[:, P * j : P * (j + 1)], id_sb
                        )
                        dst = xn2T[:, j, P * m : P * (m + 1)]
                        if j % 2 == 0:
                            nc.vector.tensor_copy(out=dst, in_=tp)
                        else:
                            nc.scalar.copy(out=dst, in_=tp)

            if dbg:
                for j in range(NT):
                    nc.gpsimd.dma_start(
                        out=dbg["d_x2"][P * j : P * (j + 1), :],
                        in_=x2[:, j, :],
                    )

        # pab closed: xnT/qT/kT/v, attT and attention scratch released.

        # ================= Phase D: FF1 =================
        pW2 = ctx.enter_context(tc.tile_pool(name="pW2", bufs=1))
        w2_sb = pW2.tile([P, NF, D], bf16)
        w2_ap = w2_d[:, :].rearrange("(k p) e -> p k e", p=P)
        for k in range(NF):
            big_dma.dma_start(out=w2_sb[:, k, :], in_=w2_ap[:, k, :])
        pH = ctx.enter_context(tc.tile_pool(name="pH", bufs=1))
        h_sb = pH.tile([P, NF, T], bf16)

        with ExitStack() as pd:
            w1p = pd.enter_context(tc.tile_pool(name="w1p", bufs=3))
            w1ap = w1_d[:, :].rearrange("(k p) e -> p k e", p=P)
            for m in range(NF):
                w1t = w1p.tile([P, ND, P], bf16, tag="w1t")
                nc.sync.dma_start(
                    out=w1t, in_=w1ap[:, :, P * m : P * (m + 1)]
                )
                for n in range(2):
                    ps = psM2.tile([P, 512], f32, tag="mm2")
                    for k in range(ND):
                        nc.tensor.matmul(
                            ps,
                            w1t[:, k, :],
                            xn2T[:, k, 512 * n : 512 * (n + 1)],
                            start=(k == 0),
                            stop=(k == ND - 1),
                        )
                    nc.scalar.activation(
                        h_sb[:, m, 512 * n : 512 * (n + 1)],
                        ps,
                        AF.Relu,
                        bias=b1_sb[:, m : m + 1],
                    )

        if dbg:
            for j in range(NF):
                nc.gpsimd.dma_start(
                    out=dbg["d_h"][P * j : P * (j + 1), :], in_=h_sb[:, j, :]
                )

        # ================= Phase E: FF2 + residual out =================
        with ExitStack() as pe:
            outp = pe.enter_context(tc.tile_pool(name="outp", bufs=3))
            for m in range(NT):
                for n in range(2):
                    ps = psM2.tile([P, 512], f32, tag="mm2")
                    for k in range(NF):
                        nc.tensor.matmul(
                            ps,
                            h_sb[:, k, P * m : P * (m + 1)],
                            w2_sb[:, k, 512 * n : 512 * (n + 1)],
                            start=(k == 0),
                            stop=(k == NF - 1),
                        )
                    ot = outp.tile([P, 512], f32, tag="ot")
                    nc.vector.tensor_add(
                        out=ot, in0=x2[:, m, 512 * n : 512 * (n + 1)], in1=ps
                    )
                    if b2_sb is not None:
                        nc.vector.tensor_add(
                            out=ot, in0=ot,
                            in1=b2_sb[:, 512 * n : 512 * (n + 1)],
                        )
                    nc.sync.dma_start(
                        out=out_d[
                            P * m : P * (m + 1), 512 * n : 512 * (n + 1)
                        ],
                        in_=ot,
                    )


def _prep_inputs(
    x, gamma1, beta1, Wq, Wk, Wv, Wp, bp, gamma2, beta2, W1, b1, W2, b2
):
    g1 = np.asarray(gamma1, np.float32)
    b1n = np.asarray(beta1, np.float32)
    g2 = np.asarray(gamma2, np.float32)
    b2n = np.asarray(beta2, np.float32)
    Wq2 = np.asarray(Wq, np.float32).transpose(1, 0, 2).reshape(D, D)
    Wk2 = np.asarray(Wk, np.float32).transpose(1, 0, 2).reshape(D, D)
    Wv2 = np.asarray(Wv, np.float32).transpose(1, 0, 2).reshape(D, D)
    W1f = np.asarray(W1, np.float32)
    qb = b1n @ Wq2
    kb = b1n @ Wk2
    vb = b1n @ Wv2
    b1p = np.asarray(b1, np.float32) + b2n @ W1f

    common = {
        "wq": np.ascontiguousarray((Wq2 * g1[:, None]).astype(BF16)),
        "wk": np.ascontiguousarray((Wk2 * g1[:, None]).astype(BF16)),
        "wv": np.ascontiguousarray((Wv2 * g1[:, None]).astype(BF16)),
        "wp": np.ascontiguousarray(np.asarray(Wp, np.float32).astype(BF16)),
        "w1": np.ascontiguousarray((W1f * g2[:, None]).astype(BF16)),
        "w2": np.ascontiguousarray(np.asarray(W2, np.float32).astype(BF16)),
        "b1t": np.ascontiguousarray(b1p.reshape(NF, P).T.astype(np.float32)),
        "qbt": np.ascontiguousarray(qb.reshape(ND, P).T.astype(np.float32)),
        "kbt": np.ascontiguousarray(kb.reshape(ND, P).T.astype(np.float32)),
        "vbr": np.ascontiguousarray(vb.reshape(1, D).astype(np.float32)),
        "mask": np.where(
            np.arange(P)[None, :] < np.arange(P)[:, None], MASKVAL, 0.0
        ).astype(np.float32),
        "ident": np.eye(P, dtype=BF16),
    }
    bpf = np.asarray(bp, np.float32)
    b2f = np.asarray(b2, np.float32)
    has_bp = bool(np.any(bpf))
    has_b2 = bool(np.any(b2f))
    if has_bp:
        common["bpr"] = np.ascontiguousarray(bpf.reshape(1, D))
    if has_b2:
        common["b2r"] = np.ascontiguousarray(b2f.reshape(1, D))
    xs = np.asarray(x, np.float32)
    return xs, common, has_bp, has_b2


def get_nc(reps=1, has_bp=False, has_b2=False, debug=False):
    key = (reps, has_bp, has_b2, debug)
    if key not in _cache:
        _cache[key] = _build(
            reps=reps, has_bp=has_bp, has_b2=has_b2, debug=debug
        )
    return _cache[key]


def run(x, common, has_bp, has_b2, reps=1, debug=False):
    from concourse.bass_utils import run_bass_kernel_spmd

    nc = get_nc(reps=reps, has_bp=has_bp, has_b2=has_b2, debug=debug)
    in_maps = [dict(common, x=np.ascontiguousarray(x[c])) for c in range(B)]
    res = run_bass_kernel_spmd(nc, in_maps, core_ids=list(range(B)))
    return res


def kernel(x, gamma1, beta1, Wq, Wk, Wv, Wp, bp, gamma2, beta2, W1, b1, W2, b2):
    xs, common, has_bp, has_b2 = _prep_inputs(
        x, gamma1, beta1, Wq, Wk, Wv, Wp, bp, gamma2, beta2, W1, b1, W2, b2
    )
    res = run(xs, common, has_bp, has_b2, reps=1)
    out = np.stack([res.results[c]["out"] for c in range(B)], axis=0)
    return out.astype(np.float32)

